# revision 36
# baseline (speedup 1.0000x reference)
"""Trainium2 Bass kernel for the PK-batch message-passing gating module.

Reference computation (per full batch of N=80 samples, 8 identities x
(5 sub=1 + 5 sub=0) samples):
  for each branch b in {sub==1, sub==0}:
    xs   = Wr_b @ x[subgroup_b]                (1x1 conv 2048 -> 256)
    aff  = per-sample gather of the 5 same-label subgroup samples,
           channel-stacked -> 1280 channels
    s_b  = relu(Wc_b @ aff)                    (1x1 conv 1280 -> 2048)
  x_fuse = sigmoid(W_f @ concat(s_i, s_v))     (1x1 conv 4096 -> 2048)
  out    = inputs * (1 + x_fuse)

All samples of one identity share the same gather, hence the same
x_fuse — so the message passing is computed once per identity and the
sigmoid gate broadcast over that identity's 10 samples.  Sharding: one
identity per NeuronCore (8 identities / 8 cores, data parallel);
weights replicated, pre-transposed/tiled on host, stored bf16.
"""

import numpy as np
import ml_dtypes

import concourse.bass as bass
import concourse.tile as tile
from concourse import bacc, mybir
from concourse.bass_utils import run_bass_kernel_spmd

N_CORES = 8
K_HALF = 5
NSAMP = 2 * K_HALF        # samples per identity
DIM = 2048
CP = 256                  # reduced channels per branch
S = 24 * 8                # spatial positions per sample
NT = DIM // 128           # 16 channel chunks of the 2048-dim axis
KC_E = (K_HALF * CP) // 128   # 10 contraction chunks for expand conv
KC_F = (2 * DIM) // 128       # 32 contraction chunks for fusion conv
BF16 = mybir.dt.bfloat16
F32 = mybir.dt.float32

_CACHE = {}


def _build():
    nc = bacc.Bacc("TRN2", target_bir_lowering=False, debug=False,
                   num_devices=N_CORES)
    x_d = nc.dram_tensor("x", [NT, 128, NSAMP * S], BF16, kind="ExternalInput")
    wr_d = nc.dram_tensor("wr", [2, 128, NT * CP], BF16, kind="ExternalInput")
    wc_d = nc.dram_tensor("wc", [2, NT, 128, KC_E * 128], BF16, kind="ExternalInput")
    wf_d = nc.dram_tensor("wf", [NT, 128, KC_F * 128], BF16, kind="ExternalInput")
    out_d = nc.dram_tensor("out", [NT, 128, NSAMP * S], F32, kind="ExternalOutput")

    AF = mybir.ActivationFunctionType
    OP = mybir.AluOpType

    with tile.TileContext(nc) as tc:
        with (
            tc.tile_pool(name="big", bufs=1) as big,
            tc.tile_pool(name="wcp", bufs=5) as wcp,
            tc.tile_pool(name="wfp", bufs=7) as wfp,
            tc.tile_pool(name="op", bufs=6) as op,
            tc.tile_pool(name="ps", bufs=8, space="PSUM") as ps,
        ):
            x_sb = big.tile([128, NT * NSAMP * S], BF16, name="x_sb", tag="x")
            # wr tiles share the wf pool ([128,4096] too, dead after R):
            # their 2 slots recycle into deeper wf prefetch during E/F
            wr_sb = [wfp.tile([128, NT * CP], BF16, name=f"wr_sb{b}", tag="wft") for b in range(2)]
            xs_sb = [big.tile([128, 2 * K_HALF * S], BF16, name=f"xs_sb{b}", tag=f"xs{b}") for b in range(2)]
            s_sb = big.tile([128, KC_F * S], BF16, name="s_sb", tag="s")
            sig_sb = big.tile([128, NT * S], BF16, name="sig_sb", tag="sig")

            # input DMAs: reduce-conv weights first (every R matmul needs
            # them), then x in kc order so R can start on partial x via
            # subtile deps; then prefetch the first fusion-weight slices so
            # the DMA queue stays busy through the PE-bound R phase.
            nc.sync.dma_start(wr_sb[0][:], wr_d[0, :, :])
            XB = 4  # x chunks per DMA
            for t in range(0, NT, XB):
                nc.sync.dma_start(
                    x_sb[:, t * NSAMP * S:(t + XB) * NSAMP * S],
                    x_d[t:t + XB, :, :].rearrange("t p c -> p t c"))
                if t == 0:
                    nc.sync.dma_start(wr_sb[1][:], wr_d[1, :, :])
            wf_tiles = {}
            for k in range(5):
                wft = wfp.tile([128, KC_F * 128], BF16, name="wft", tag="wft")
                nc.sync.dma_start(wft[:], wf_d[k, :, :])
                wf_tiles[k] = wft

            # PE warm-up: dummy matmuls keep the PE HAM busy through the
            # initial x/weight DMA so stage R starts at 2.4 GHz. Results are
            # discarded.
            warm = big.tile([128, 512], BF16, name="warm", tag="warm")
            wpt = ps.tile([128, 512], F32, name="wpt", tag="pt")
            nc.vector.memset(warm[:], 0.0)
            for i in range(12):
                nc.tensor.matmul(wpt[:], warm[:, 0:128], warm[:, 0:512],
                                 start=(i == 0), stop=(i == 11))

            # Stage R: reduce conv, xs = Wr @ x.  kc-OUTER with exactly 8
            # live psum groups of N=480 (2.5 samples each) so the PE consumes
            # each x channel-chunk the moment its DMA lands — R streams with
            # the x load instead of waiting for all of it.
            NH = K_HALF * S // 2          # 480 cols per psum group
            rpt = {}
            for b in range(2):
                for mc in range(2):
                    for h in range(2):
                        rpt[(b, mc, h)] = ps.tile([128, NH], F32, name="pt", tag="pt")
            for kc in range(NT):
                for b in range(2):
                    for mc in range(2):
                        for h in range(2):
                            col = kc * NSAMP * S + b * K_HALF * S + h * NH
                            nc.tensor.matmul(
                                rpt[(b, mc, h)][:],
                                wr_sb[b][:, kc * CP + mc * 128: kc * CP + (mc + 1) * 128],
                                x_sb[:, col: col + NH],
                                start=(kc == 0), stop=(kc == NT - 1))
            # scatter psum cols (2.5 samples per group) into xs channel-stack
            for b in range(2):
                for mc in range(2):
                    for h in range(2):
                        base_col = h * NH          # within branch b's 5 samples
                        off = 0
                        while off < NH:
                            j = (base_col + off) // S
                            joff = (base_col + off) % S
                            seg = min(S - joff, NH - off)
                            nc.scalar.activation(
                                xs_sb[b][:, (2 * j + mc) * S + joff:
                                       (2 * j + mc) * S + joff + seg],
                                rpt[(b, mc, h)][:, off:off + seg],
                                AF.Copy)
                            off += seg

            # Stage E: expand conv, s = relu(Wc @ xs-stack); wc loaded in
            # paired slices for bigger DMAs
            for b in range(2):
                for mc0 in range(0, NT, 2):
                    wct = wcp.tile([128, 2 * KC_E * 128], BF16, name="wct", tag="wct")
                    nc.sync.dma_start(
                        wct[:],
                        wc_d[b, mc0:mc0 + 2, :, :].rearrange("t p c -> p t c"))
                    for mi in range(2):
                        mc = mc0 + mi
                        pt = ps.tile([128, S], F32, name="pt", tag="pt")
                        for kc in range(KC_E):
                            nc.tensor.matmul(
                                pt[:],
                                wct[:, (mi * KC_E + kc) * 128:(mi * KC_E + kc + 1) * 128],
                                xs_sb[b][:, kc * S:(kc + 1) * S],
                                start=(kc == 0), stop=(kc == KC_E - 1))
                        nc.scalar.activation(
                            s_sb[:, (b * NT + mc) * S:(b * NT + mc + 1) * S], pt[:],
                            AF.Relu)

            # Stage F+O: fusion conv + sigmoid, then out = x * (1 + sig).
            # Bulk out DMAs go on the second HWDGE ring (scalar) so they are
            # not FIFO-blocked behind the wf loads on the sync ring; the
            # last few go on the by-then-idle sync ring to burst the tail.
            for mc in range(NT):
                if mc in wf_tiles:
                    wft = wf_tiles[mc]
                else:
                    wft = wfp.tile([128, KC_F * 128], BF16, name="wft", tag="wft")
                    nc.sync.dma_start(wft[:], wf_d[mc, :, :])
                pt = ps.tile([128, S], F32, name="pt", tag="pt")
                for kc in range(KC_F):
                    nc.tensor.matmul(
                        pt[:],
                        wft[:, kc * 128:(kc + 1) * 128],
                        s_sb[:, kc * S:(kc + 1) * S],
                        start=(kc == 0), stop=(kc == KC_F - 1))
                nc.scalar.activation(sig_sb[:, mc * S:(mc + 1) * S], pt[:], AF.Sigmoid)
                JH = NSAMP // 2
                for half in range(2):
                    ot = op.tile([128, JH * S], F32, name="ot", tag="ot")
                    xs_ap = x_sb[:, mc * NSAMP * S + half * JH * S:
                                 mc * NSAMP * S + (half + 1) * JH * S]
                    sig_b, x_b = bass.broadcast_tensor_aps(
                        sig_sb[:, mc * S:(mc + 1) * S].rearrange("p (j s) -> p j s", j=1),
                        xs_ap.rearrange("p (j s) -> p j s", j=JH))
                    nc.vector.scalar_tensor_tensor(
                        ot[:].rearrange("p (j s) -> p j s", j=JH),
                        sig_b, 1.0, x_b, OP.add, OP.mult)
                    out_eng = nc.scalar if mc < 12 else nc.sync
                    out_eng.dma_start(
                        out_d[mc, :, half * JH * S:(half + 1) * JH * S],
                        ot[:])

    nc.compile()
    return nc


def _get_nc():
    if "nc" not in _CACHE:
        _CACHE["nc"] = _build()
    return _CACHE["nc"]


def _prep_weights(W_ri, W_rv, W_ci, W_cv, W_f):
    bf = ml_dtypes.bfloat16
    # wr[b][p, kc*CP + m] = W_r[m, kc*128 + p]
    wr = np.stack([
        np.ascontiguousarray(
            W.T.reshape(NT, 128, CP).transpose(1, 0, 2).reshape(128, NT * CP))
        for W in (W_ri, W_rv)
    ]).astype(bf)
    # wc[b][mc][p][kc*128+m] = W_c[mc*128+m, kc*128+p]
    wc = np.stack([
        np.ascontiguousarray(
            W.reshape(NT, 128, KC_E, 128).transpose(0, 3, 2, 1).reshape(NT, 128, KC_E * 128))
        for W in (W_ci, W_cv)
    ]).astype(bf)
    # wf[mc][p][kc*128+m] = W_f[mc*128+m, kc*128+p]
    wf = np.ascontiguousarray(
        W_f.reshape(NT, 128, KC_F, 128).transpose(0, 3, 2, 1).reshape(NT, 128, KC_F * 128)
    ).astype(bf)
    return wr, wc, wf


def kernel(inputs, labels, sub, W_ri, W_rv, W_ci, W_cv, W_f):
    inputs = np.asarray(inputs, dtype=np.float32)
    labels = np.asarray(labels)
    sub = np.asarray(sub)
    W_ri = np.asarray(W_ri, dtype=np.float32)
    W_rv = np.asarray(W_rv, dtype=np.float32)
    W_ci = np.asarray(W_ci, dtype=np.float32)
    W_cv = np.asarray(W_cv, dtype=np.float32)
    W_f = np.asarray(W_f, dtype=np.float32)

    n, c, h, w = inputs.shape
    assert (n, c, h * w) == (N_CORES * NSAMP, DIM, S)
    x = inputs.reshape(n, c, h * w)

    # identity groups: all samples of one label share the same gather set
    uniq = np.unique(labels)
    assert len(uniq) == N_CORES, f"expected {N_CORES} identities, got {len(uniq)}"
    order = []
    for g in uniq:
        idx = np.nonzero(labels == g)[0]
        i_s = [int(i) for i in idx if sub[i] == 1]
        v_s = [int(i) for i in idx if sub[i] == 0]
        assert len(i_s) == K_HALF and len(v_s) == K_HALF, \
            f"identity {g}: {len(i_s)}/{len(v_s)} split not {K_HALF}/{K_HALF}"
        order.append(i_s + v_s)

    wr, wc, wf = _prep_weights(W_ri, W_rv, W_ci, W_cv, W_f)

    bf = ml_dtypes.bfloat16
    in_maps = []
    for g in range(N_CORES):
        xg = x[order[g]]                                      # [10, 2048, 192]
        xt = np.ascontiguousarray(
            xg.reshape(NSAMP, NT, 128, S).transpose(1, 2, 0, 3)
              .reshape(NT, 128, NSAMP * S)).astype(bf)
        in_maps.append({"x": xt, "wr": wr, "wc": wc, "wf": wf})

    nc = _get_nc()
    res = run_bass_kernel_spmd(nc, in_maps, core_ids=list(range(N_CORES)))

    out = np.empty_like(inputs)
    for g in range(N_CORES):
        og = res.results[g]["out"]                            # [16, 128, 1920] f32
        og = og.reshape(NT, 128, NSAMP, S).transpose(2, 0, 1, 3).reshape(NSAMP, c, h, w)
        out[order[g]] = og
    return out


# revision 37
# speedup vs baseline: 1.0109x; 1.0109x over previous
"""Trainium2 Bass kernel for the PK-batch message-passing gating module.

Reference computation (per full batch of N=80 samples, 8 identities x
(5 sub=1 + 5 sub=0) samples):
  for each branch b in {sub==1, sub==0}:
    xs   = Wr_b @ x[subgroup_b]                (1x1 conv 2048 -> 256)
    aff  = per-sample gather of the 5 same-label subgroup samples,
           channel-stacked -> 1280 channels
    s_b  = relu(Wc_b @ aff)                    (1x1 conv 1280 -> 2048)
  x_fuse = sigmoid(W_f @ concat(s_i, s_v))     (1x1 conv 4096 -> 2048)
  out    = inputs * (1 + x_fuse)

All samples of one identity share the same gather, hence the same
x_fuse — so the message passing is computed once per identity and the
sigmoid gate broadcast over that identity's 10 samples.  Sharding: one
identity per NeuronCore (8 identities / 8 cores, data parallel);
weights replicated, pre-transposed/tiled on host, stored bf16.
"""

import numpy as np
import ml_dtypes

import concourse.bass as bass
import concourse.tile as tile
from concourse import bacc, mybir
from concourse.bass_utils import run_bass_kernel_spmd

N_CORES = 8
K_HALF = 5
NSAMP = 2 * K_HALF        # samples per identity
DIM = 2048
CP = 256                  # reduced channels per branch
S = 24 * 8                # spatial positions per sample
NT = DIM // 128           # 16 channel chunks of the 2048-dim axis
KC_E = (K_HALF * CP) // 128   # 10 contraction chunks for expand conv
KC_F = (2 * DIM) // 128       # 32 contraction chunks for fusion conv
BF16 = mybir.dt.bfloat16
F32 = mybir.dt.float32

_CACHE = {}


def _build():
    nc = bacc.Bacc("TRN2", target_bir_lowering=False, debug=False,
                   num_devices=N_CORES)
    x_d = nc.dram_tensor("x", [NT, 128, NSAMP * S], BF16, kind="ExternalInput")
    wr_d = nc.dram_tensor("wr", [2, 128, NT * CP], BF16, kind="ExternalInput")
    wc_d = nc.dram_tensor("wc", [2, NT, 128, KC_E * 128], BF16, kind="ExternalInput")
    wf_d = nc.dram_tensor("wf", [NT, 128, KC_F * 128], BF16, kind="ExternalInput")
    out_d = nc.dram_tensor("out", [NT, 128, NSAMP * S], F32, kind="ExternalOutput")

    AF = mybir.ActivationFunctionType
    OP = mybir.AluOpType

    with tile.TileContext(nc) as tc:
        with (
            tc.tile_pool(name="big", bufs=1) as big,
            tc.tile_pool(name="wcp", bufs=5) as wcp,
            tc.tile_pool(name="wfp", bufs=7) as wfp,
            tc.tile_pool(name="op", bufs=6) as op,
            tc.tile_pool(name="ps", bufs=8, space="PSUM") as ps,
        ):
            x_sb = big.tile([128, NT * NSAMP * S], BF16, name="x_sb", tag="x")
            # wr tiles share the wf pool ([128,4096] too, dead after R):
            # their 2 slots recycle into deeper wf prefetch during E/F
            wr_sb = [wfp.tile([128, NT * CP], BF16, name=f"wr_sb{b}", tag="wft") for b in range(2)]
            xs_sb = [big.tile([128, 2 * K_HALF * S], BF16, name=f"xs_sb{b}", tag=f"xs{b}") for b in range(2)]
            s_sb = big.tile([128, KC_F * S], BF16, name="s_sb", tag="s")
            sig_sb = big.tile([128, NT * S], BF16, name="sig_sb", tag="sig")

            # input DMAs: reduce-conv weights first (every R matmul needs
            # them), then x in kc order so R can start on partial x via
            # subtile deps; then prefetch the first fusion-weight slices so
            # the DMA queue stays busy through the PE-bound R phase.
            nc.sync.dma_start(wr_sb[0][:], wr_d[0, :, :])
            XB = 4  # x chunks per DMA
            for t in range(0, NT, XB):
                nc.sync.dma_start(
                    x_sb[:, t * NSAMP * S:(t + XB) * NSAMP * S],
                    x_d[t:t + XB, :, :].rearrange("t p c -> p t c"))
                if t == 0:
                    nc.sync.dma_start(wr_sb[1][:], wr_d[1, :, :])
            wf_tiles = {}
            for k in range(5):
                wft = wfp.tile([128, KC_F * 128], BF16, name="wft", tag="wft")
                nc.sync.dma_start(wft[:], wf_d[k, :, :])
                wf_tiles[k] = wft

            # PE warm-up: dummy matmuls keep the PE HAM busy through the
            # initial x/weight DMA so stage R starts at 2.4 GHz. Results are
            # discarded.
            warm = big.tile([128, 512], BF16, name="warm", tag="warm")
            wpt = ps.tile([128, 512], F32, name="wpt", tag="pt")
            nc.vector.memset(warm[:], 0.0)
            for i in range(12):
                nc.tensor.matmul(wpt[:], warm[:, 0:128], warm[:, 0:512],
                                 start=(i == 0), stop=(i == 11))

            # Stage R: reduce conv, xs = Wr @ x.  kc-OUTER with exactly 8
            # live psum groups of N=480 (2.5 samples each) so the PE consumes
            # each x channel-chunk the moment its DMA lands — R streams with
            # the x load instead of waiting for all of it.
            NH = K_HALF * S // 2          # 480 cols per psum group
            rpt = {}
            for b in range(2):
                for mc in range(2):
                    for h in range(2):
                        rpt[(b, mc, h)] = ps.tile([128, NH], F32, name="pt", tag="pt")
            for kc in range(NT):
                for b in range(2):
                    for mc in range(2):
                        for h in range(2):
                            col = kc * NSAMP * S + b * K_HALF * S + h * NH
                            nc.tensor.matmul(
                                rpt[(b, mc, h)][:],
                                wr_sb[b][:, kc * CP + mc * 128: kc * CP + (mc + 1) * 128],
                                x_sb[:, col: col + NH],
                                start=(kc == 0), stop=(kc == NT - 1))
            # scatter psum cols (2.5 samples per group) into xs channel-stack
            for b in range(2):
                for mc in range(2):
                    for h in range(2):
                        base_col = h * NH          # within branch b's 5 samples
                        off = 0
                        while off < NH:
                            j = (base_col + off) // S
                            joff = (base_col + off) % S
                            seg = min(S - joff, NH - off)
                            nc.scalar.activation(
                                xs_sb[b][:, (2 * j + mc) * S + joff:
                                       (2 * j + mc) * S + joff + seg],
                                rpt[(b, mc, h)][:, off:off + seg],
                                AF.Copy)
                            off += seg

            # Stage E: expand conv, s = relu(Wc @ xs-stack); wc loaded in
            # paired slices for bigger DMAs
            for b in range(2):
                for mc in range(NT):
                    wct = wcp.tile([128, KC_E * 128], BF16, name="wct", tag="wct",
                                   bufs=10)
                    nc.sync.dma_start(wct[:], wc_d[b, mc, :, :])
                    pt = ps.tile([128, S], F32, name="pt", tag="pt")
                    for kc in range(KC_E):
                        nc.tensor.matmul(
                            pt[:],
                            wct[:, kc * 128:(kc + 1) * 128],
                            xs_sb[b][:, kc * S:(kc + 1) * S],
                            start=(kc == 0), stop=(kc == KC_E - 1))
                    nc.scalar.activation(
                        s_sb[:, (b * NT + mc) * S:(b * NT + mc + 1) * S], pt[:],
                        AF.Relu)

            # Stage F+O: fusion conv + sigmoid, then out = x * (1 + sig).
            # Bulk out DMAs go on the second HWDGE ring (scalar) so they are
            # not FIFO-blocked behind the wf loads on the sync ring; the
            # last few go on the by-then-idle sync ring to burst the tail.
            for mc in range(NT):
                if mc in wf_tiles:
                    wft = wf_tiles[mc]
                else:
                    wft = wfp.tile([128, KC_F * 128], BF16, name="wft", tag="wft")
                    nc.sync.dma_start(wft[:], wf_d[mc, :, :])
                pt = ps.tile([128, S], F32, name="pt", tag="pt")
                for kc in range(KC_F):
                    nc.tensor.matmul(
                        pt[:],
                        wft[:, kc * 128:(kc + 1) * 128],
                        s_sb[:, kc * S:(kc + 1) * S],
                        start=(kc == 0), stop=(kc == KC_F - 1))
                nc.scalar.activation(sig_sb[:, mc * S:(mc + 1) * S], pt[:], AF.Sigmoid)
                JH = NSAMP // 2
                for half in range(2):
                    ot = op.tile([128, JH * S], F32, name="ot", tag="ot")
                    xs_ap = x_sb[:, mc * NSAMP * S + half * JH * S:
                                 mc * NSAMP * S + (half + 1) * JH * S]
                    sig_b, x_b = bass.broadcast_tensor_aps(
                        sig_sb[:, mc * S:(mc + 1) * S].rearrange("p (j s) -> p j s", j=1),
                        xs_ap.rearrange("p (j s) -> p j s", j=JH))
                    nc.vector.scalar_tensor_tensor(
                        ot[:].rearrange("p (j s) -> p j s", j=JH),
                        sig_b, 1.0, x_b, OP.add, OP.mult)
                    out_eng = nc.scalar if mc < 12 else nc.sync
                    out_eng.dma_start(
                        out_d[mc, :, half * JH * S:(half + 1) * JH * S],
                        ot[:])

    nc.compile()
    return nc


def _get_nc():
    if "nc" not in _CACHE:
        _CACHE["nc"] = _build()
    return _CACHE["nc"]


def _prep_weights(W_ri, W_rv, W_ci, W_cv, W_f):
    bf = ml_dtypes.bfloat16
    # wr[b][p, kc*CP + m] = W_r[m, kc*128 + p]
    wr = np.stack([
        np.ascontiguousarray(
            W.T.reshape(NT, 128, CP).transpose(1, 0, 2).reshape(128, NT * CP))
        for W in (W_ri, W_rv)
    ]).astype(bf)
    # wc[b][mc][p][kc*128+m] = W_c[mc*128+m, kc*128+p]
    wc = np.stack([
        np.ascontiguousarray(
            W.reshape(NT, 128, KC_E, 128).transpose(0, 3, 2, 1).reshape(NT, 128, KC_E * 128))
        for W in (W_ci, W_cv)
    ]).astype(bf)
    # wf[mc][p][kc*128+m] = W_f[mc*128+m, kc*128+p]
    wf = np.ascontiguousarray(
        W_f.reshape(NT, 128, KC_F, 128).transpose(0, 3, 2, 1).reshape(NT, 128, KC_F * 128)
    ).astype(bf)
    return wr, wc, wf


def kernel(inputs, labels, sub, W_ri, W_rv, W_ci, W_cv, W_f):
    inputs = np.asarray(inputs, dtype=np.float32)
    labels = np.asarray(labels)
    sub = np.asarray(sub)
    W_ri = np.asarray(W_ri, dtype=np.float32)
    W_rv = np.asarray(W_rv, dtype=np.float32)
    W_ci = np.asarray(W_ci, dtype=np.float32)
    W_cv = np.asarray(W_cv, dtype=np.float32)
    W_f = np.asarray(W_f, dtype=np.float32)

    n, c, h, w = inputs.shape
    assert (n, c, h * w) == (N_CORES * NSAMP, DIM, S)
    x = inputs.reshape(n, c, h * w)

    # identity groups: all samples of one label share the same gather set
    uniq = np.unique(labels)
    assert len(uniq) == N_CORES, f"expected {N_CORES} identities, got {len(uniq)}"
    order = []
    for g in uniq:
        idx = np.nonzero(labels == g)[0]
        i_s = [int(i) for i in idx if sub[i] == 1]
        v_s = [int(i) for i in idx if sub[i] == 0]
        assert len(i_s) == K_HALF and len(v_s) == K_HALF, \
            f"identity {g}: {len(i_s)}/{len(v_s)} split not {K_HALF}/{K_HALF}"
        order.append(i_s + v_s)

    wr, wc, wf = _prep_weights(W_ri, W_rv, W_ci, W_cv, W_f)

    bf = ml_dtypes.bfloat16
    in_maps = []
    for g in range(N_CORES):
        xg = x[order[g]]                                      # [10, 2048, 192]
        xt = np.ascontiguousarray(
            xg.reshape(NSAMP, NT, 128, S).transpose(1, 2, 0, 3)
              .reshape(NT, 128, NSAMP * S)).astype(bf)
        in_maps.append({"x": xt, "wr": wr, "wc": wc, "wf": wf})

    nc = _get_nc()
    res = run_bass_kernel_spmd(nc, in_maps, core_ids=list(range(N_CORES)))

    out = np.empty_like(inputs)
    for g in range(N_CORES):
        og = res.results[g]["out"]                            # [16, 128, 1920] f32
        og = og.reshape(NT, 128, NSAMP, S).transpose(2, 0, 1, 3).reshape(NSAMP, c, h, w)
        out[order[g]] = og
    return out
